# revision 10
# baseline (speedup 1.0000x reference)
"""Trainium2 Bass kernel for nn_CrossAttention_28183575396415.

The reference block-mask gives every query exactly one key (kv = q_idx // 3),
so the softmax weight is identically 1 and the q/k projections, RMSNorm and
RoPE are dead code.  The module reduces to

    out[b, t] = x_kv[b, t // 3] @ Wv.T @ Wproj.T
              = x_kv[b, t // 3] @ WfT          with WfT = Wv.T @ Wproj.T

Strategy (8 NeuronCores, SPMD):
  - Host folds the two projection matrices into WfT (float64 accumulate,
    stored bf16) and row-shards the 4*2048 = 8192 kv rows 8 ways (1024
    rows/core).  All device IO is bf16: 4 MiB in + 6 MiB out per core
    (vs 20 MiB for fp32), which moves the kernel from DMA-bound to the
    PE roofline.  rel_l2 error stays ~4e-3, far inside the 2e-2 gate.
  - Device pipeline: the 1024 shard rows are processed as 8 row-blocks of
    128.  Per block: 8 accumulating matmuls per PSUM column-half
    (lhsT = x.T k-tile, stationary; rhs = WfT k-tile, moving), eviction
    PSUM->SBUF with fp32->bf16 downcast, and 3 output DMAs (the t//3
    replication).  Dependencies are block-local, so input DMA, PE, evict
    and output DMA all stream concurrently -- no global barrier.
  - Host unshard = concatenate the 8 bf16 [3072, 1024] slices, upcast to
    fp32.
"""

import json

import numpy as np
import ml_dtypes

import concourse.bass as bass
import concourse.mybir as mybir
from concourse.tile import TileContext
from concourse.vector_clock import ScopedClock
from concourse.bass_utils import run_bass_kernel_spmd

P = 128          # partitions
C = 1024         # model dim
T_K = C // P     # contraction k-tiles
R_B = 8          # row blocks per core (1024 rows / 128)
N = 512          # matmul free dim (one PSUM bank of fp32)
L = 3            # replication factor (Tq // Tkv)
ROWS_PER_CORE = 1024
N_CORES = 8


class SlimTailTileContext(TileContext):
    """Tile's kernel tail is drain -> barrier -> ~280 serialized per-semaphore
    clear instructions -> barrier (~8 us measured).  The clears only matter if
    the loaded NEFF executes more than once; every kernel() call here builds a
    fresh jit executable (fresh NEFF load, semaphores re-initialized), so skip
    them and the second barrier.  The drain still waits for every DMA queue,
    so outputs are complete before the program ends."""

    def _drain_and_barrier(self, tick_clock, wait_clock):
        # The SP drain (with its hoisted wait chain) already gates on every
        # engine's clock and every DMA queue, so outputs are complete when SP
        # retires; with no sem-clears to order, the closing all-engine
        # barrier adds nothing but latency.
        drain_inst = self.nc.sync.drain()
        wait_clock.add_sem_waits(
            drain_inst.ins, ScopedClock({None: tick_clock.global_clock})
        )
        popped = self.nc._tile_sem_poison_stack.pop()
        assert popped is self._sem_poison

def _split_multiwaits(nc: bass.Bass) -> None:
    """This container's walrus allows only ONE sync-wait on several
    instruction formats (Drain/CTRL, Matmult's LDWEIGHTS half, ...).  Tile
    can emit more.  Post-pass the serialized BIR: for any instruction with
    >1 on_wait, hoist all but the last wait onto single-wait EventSemaphore
    carriers inserted immediately before it on the same engine (waits then
    execute in queue order — semantics unchanged).  The patched JSON is
    pinned on the instance so every downstream serialization sees it."""
    raw = bass.Bass.to_json_bytes(nc)
    j = json.loads(raw)
    n_hoisted = 0
    for f in j["functions"]:
        for bb in f["blocks"]:
            new_insts = []
            for ins in bb["instructions"]:
                si = ins.get("sync_info")
                waits = si.get("on_wait", []) if si else []
                if len(waits) > 1:
                    for i, w in enumerate(waits[:-1]):
                        carrier = {
                            "engine": ins["engine"],
                            "ins": [],
                            "outs": [],
                            "name": f"{ins['name']}_hw{i}",
                            "opcode": "EventSemaphore",
                            "sync_info": {"on_update": [], "on_wait": [w]},
                        }
                        if "debug" in ins:
                            carrier["debug"] = ins["debug"]
                        new_insts.append(carrier)
                        n_hoisted += 1
                    si["on_wait"] = waits[-1:]
                new_insts.append(ins)
            bb["instructions"] = new_insts
    patched = json.dumps(j).encode()
    nc.to_json_bytes = lambda: patched


def _build() -> bass.Bass:
    nc = bass.Bass("TRN2")
    bf16 = mybir.dt.bfloat16

    # xb[r*128 + kp, t*128 + row] = x_shard[r*128 + row, t*128 + kp]
    # i.e. per row-block r, the 8 stationary k-tiles side by side.
    xb = nc.dram_tensor("xb", [ROWS_PER_CORE, C], bf16, kind="ExternalInput")
    # wd = WfT  [k, c]
    wd = nc.dram_tensor("wd", [C, C], bf16, kind="ExternalInput")
    out = nc.dram_tensor(
        "out", [L * ROWS_PER_CORE, C], bf16, kind="ExternalOutput"
    )
    # out row (3*g + rep) <- z row g
    out_rep = out.rearrange("(g r) c -> g r c", r=L)  # [1024, 3, 1024]

    with SlimTailTileContext(nc) as tc:
        with (
            tc.tile_pool(name="xbp", bufs=1) as xb_pool,
            tc.tile_pool(name="wp", bufs=1) as w_pool,
            tc.tile_pool(name="psum", bufs=8, space="PSUM") as psum_pool,
            tc.tile_pool(name="zout", bufs=4) as z_pool,
            tc.tile_pool(name="warm", bufs=1) as warm_pool,
        ):
            # Only SP / ACT (HWDGE) and gpsimd (SWDGE) can trigger DMAs, and
            # a DGE queue runs its transfers serially, so the W stream (which
            # the PE consumes at ~0.5 us/tile during block 0) is split over
            # two queues in 0.5 MiB pair-DMAs while scalar streams the x
            # row-blocks (needed only every ~4 us).
            wt = [
                w_pool.tile([P, C], bf16, name=f"w{t}", tag=f"w{t}")
                for t in range(T_K)
            ]
            xbt = [
                xb_pool.tile([P, C], bf16, name=f"xb{r}", tag=f"xb{r}")
                for r in range(R_B)
            ]

            def w_dma(eng, t):
                eng.dma_start(wt[t][:], wd[t * P : (t + 1) * P, :])

            def xb_dma(eng, r):
                eng.dma_start(xbt[r][:], xb[r * P : (r + 1) * P, :])

            # Two HWDGE queues (SP, ACT) at ~165 GB/s each, serial per
            # queue; per-tile DMAs so a matmul waits only on the one tile
            # it reads.  W k-tiles alternate queues so they land close to
            # the PE's consumption order; xb0 leads scalar, xb1 rides the
            # (slow but off-critical-path) gpsimd SWDGE, later xb tiles
            # trail the W stream on both queues.
            w_dma(nc.sync, 0)
            xb_dma(nc.scalar, 0)
            xb_dma(nc.gpsimd, 1)
            w_dma(nc.sync, 2)
            w_dma(nc.scalar, 1)
            w_dma(nc.sync, 4)
            w_dma(nc.scalar, 3)
            w_dma(nc.sync, 6)
            w_dma(nc.scalar, 5)
            w_dma(nc.scalar, 7)
            for i, r in enumerate(range(2, R_B)):
                xb_dma([nc.sync, nc.scalar][i % 2], r)

            out_eng = [nc.sync, nc.scalar, nc.gpsimd]
            # gpsimd cannot access PSUM; DVE + ACT can (different banks ok)
            evict_eng = [nc.vector.tensor_copy, nc.scalar.copy]
            for r in range(R_B):
                ps = [
                    psum_pool.tile([P, N], mybir.dt.float32, name=f"ps{r}_{h}", tag="ps")
                    for h in range(2)
                ]
                z = z_pool.tile([P, C], bf16, name=f"z{r}", tag="z")
                last = r == R_B - 1
                # h-outer: all 8 k-matmuls of column-half 0, then half 1 --
                # half 0 can evict + start its output DMAs while the PE is
                # still on half 1, which shortens the kernel tail.
                for h in range(2):
                    for t in range(T_K):
                        nc.tensor.matmul(
                            ps[h][:],
                            xbt[r][:, t * P : (t + 1) * P],
                            wt[t][:, h * N : (h + 1) * N],
                            start=(t == 0),
                            stop=(t == T_K - 1),
                        )
                    evict_eng[h](z[:, h * N : (h + 1) * N], ps[h][:])
                    if last:
                        # split the final block's outputs by half so the
                        # h0 transfers run during the h1 matmuls, and keep
                        # them off the slow gpsimd SWDGE queue
                        for rep in range(L):
                            [nc.sync, nc.scalar, nc.sync if h else nc.scalar][rep].dma_start(
                                out_rep[r * P : (r + 1) * P, rep, h * N : (h + 1) * N],
                                z[:, h * N : (h + 1) * N],
                            )
                if not last:
                    for rep in range(L):
                        out_eng[(r * L + rep) % 3].dma_start(
                            out_rep[r * P : (r + 1) * P, rep, :], z[:]
                        )

    _split_multiwaits(nc)
    return nc


_NC_CACHE: dict = {}


def _get_nc() -> bass.Bass:
    if "nc" not in _NC_CACHE:
        _NC_CACHE["nc"] = _build()
    return _NC_CACHE["nc"]


def kernel(x_q, x_kv, Wq, Wk, Wv, Wproj):
    B, Tkv, C_ = x_kv.shape
    assert (B, Tkv, C_) == (4, 2048, C)

    # Fold the two projections: z = x @ Wv.T @ Wproj.T = x @ WfT
    WfT = (Wv.astype(np.float64).T @ Wproj.astype(np.float64).T).astype(
        ml_dtypes.bfloat16
    )

    x_flat = x_kv.reshape(B * Tkv, C).astype(ml_dtypes.bfloat16)
    in_maps = []
    for c in range(N_CORES):
        shard = x_flat[c * ROWS_PER_CORE : (c + 1) * ROWS_PER_CORE]
        # [r, row, t, kp] -> [r, kp, t, row]: per row-block, k on partitions
        xb = np.ascontiguousarray(
            shard.reshape(R_B, P, T_K, P).transpose(0, 3, 2, 1)
        ).reshape(ROWS_PER_CORE, C)
        in_maps.append({"xb": xb, "wd": WfT})

    nc = _get_nc()
    res = run_bass_kernel_spmd(nc, in_maps, core_ids=list(range(N_CORES)))

    Tq = L * Tkv
    out_flat = np.concatenate(
        [res.results[c]["out"].astype(np.float32) for c in range(N_CORES)], axis=0
    )  # [B*Tq, C]
    return out_flat.reshape(B, Tq, C)


# revision 12
# speedup vs baseline: 1.1624x; 1.1624x over previous
"""Trainium2 Bass kernel for nn_CrossAttention_28183575396415.

The reference block-mask gives every query exactly one key (kv = q_idx // 3),
so the softmax weight is identically 1 and the q/k projections, RMSNorm and
RoPE are dead code.  The module reduces to

    out[b, t] = x_kv[b, t // 3] @ Wv.T @ Wproj.T
              = x_kv[b, t // 3] @ WfT          with WfT = Wv.T @ Wproj.T

Strategy (8 NeuronCores, SPMD):
  - Host folds the two projection matrices into WfT (float64 accumulate,
    stored bf16) and row-shards the 4*2048 = 8192 kv rows 8 ways (1024
    rows/core).  All device IO is bf16: 4 MiB in + 6 MiB out per core
    (vs 20 MiB for fp32), which moves the kernel from DMA-bound to the
    PE roofline.  rel_l2 error stays ~4e-3, far inside the 2e-2 gate.
  - Device pipeline: the 1024 shard rows are processed as 8 row-blocks of
    128.  Per block: 8 accumulating matmuls per PSUM column-half
    (lhsT = x.T k-tile, stationary; rhs = WfT k-tile, moving), eviction
    PSUM->SBUF with fp32->bf16 downcast, and 3 output DMAs (the t//3
    replication).  Dependencies are block-local, so input DMA, PE, evict
    and output DMA all stream concurrently -- no global barrier.
  - Host unshard = concatenate the 8 bf16 [3072, 1024] slices, upcast to
    fp32.
"""

import json

import numpy as np
import ml_dtypes

import concourse.bass as bass
import concourse.mybir as mybir
from concourse.tile import TileContext
from concourse.vector_clock import ScopedClock
from concourse.bass_utils import run_bass_kernel_spmd

P = 128          # partitions
C = 1024         # model dim
T_K = C // P     # contraction k-tiles
R_B = 8          # row blocks per core (1024 rows / 128)
N = 512          # matmul free dim (one PSUM bank of fp32)
L = 3            # replication factor (Tq // Tkv)
ROWS_PER_CORE = 1024
N_CORES = 8


class SlimTailTileContext(TileContext):
    """Tile's kernel tail is drain -> barrier -> ~280 serialized per-semaphore
    clear instructions -> barrier (~8 us measured).  The clears only matter if
    the loaded NEFF executes more than once; every kernel() call here builds a
    fresh jit executable (fresh NEFF load, semaphores re-initialized), so skip
    them and the second barrier.  The drain still waits for every DMA queue,
    so outputs are complete before the program ends."""

    def _drain_and_barrier(self, tick_clock, wait_clock):
        # The SP drain (with its hoisted wait chain) already gates on every
        # engine's clock and every DMA queue, so outputs are complete when SP
        # retires; with no sem-clears to order, the closing all-engine
        # barrier adds nothing but latency.
        drain_inst = self.nc.sync.drain()
        wait_clock.add_sem_waits(
            drain_inst.ins, ScopedClock({None: tick_clock.global_clock})
        )
        popped = self.nc._tile_sem_poison_stack.pop()
        assert popped is self._sem_poison

def _split_multiwaits(nc: bass.Bass) -> None:
    """This container's walrus allows only ONE sync-wait on several
    instruction formats (Drain/CTRL, Matmult's LDWEIGHTS half, ...).  Tile
    can emit more.  Post-pass the serialized BIR: for any instruction with
    >1 on_wait, hoist all but the last wait onto single-wait EventSemaphore
    carriers inserted immediately before it on the same engine (waits then
    execute in queue order — semantics unchanged).  The patched JSON is
    pinned on the instance so every downstream serialization sees it."""
    raw = bass.Bass.to_json_bytes(nc)
    j = json.loads(raw)
    n_hoisted = 0
    for f in j["functions"]:
        for bb in f["blocks"]:
            new_insts = []
            for ins in bb["instructions"]:
                si = ins.get("sync_info")
                waits = si.get("on_wait", []) if si else []
                if len(waits) > 1:
                    for i, w in enumerate(waits[:-1]):
                        carrier = {
                            "engine": ins["engine"],
                            "ins": [],
                            "outs": [],
                            "name": f"{ins['name']}_hw{i}",
                            "opcode": "EventSemaphore",
                            "sync_info": {"on_update": [], "on_wait": [w]},
                        }
                        if "debug" in ins:
                            carrier["debug"] = ins["debug"]
                        new_insts.append(carrier)
                        n_hoisted += 1
                    si["on_wait"] = waits[-1:]
                new_insts.append(ins)
            bb["instructions"] = new_insts
    patched = json.dumps(j).encode()
    nc.to_json_bytes = lambda: patched


def _build() -> bass.Bass:
    nc = bass.Bass("TRN2")
    bf16 = mybir.dt.bfloat16

    # xb[r*128 + kp, t*128 + row] = x_shard[r*128 + row, t*128 + kp]
    # i.e. per row-block r, the 8 stationary k-tiles side by side.
    xb = nc.dram_tensor("xb", [ROWS_PER_CORE, C], bf16, kind="ExternalInput")
    # wd = WfT  [k, c]
    wd = nc.dram_tensor("wd", [C, C], bf16, kind="ExternalInput")
    out = nc.dram_tensor(
        "out", [L * ROWS_PER_CORE, C], bf16, kind="ExternalOutput"
    )
    # out row (3*g + rep) <- z row g
    out_rep = out.rearrange("(g r) c -> g r c", r=L)  # [1024, 3, 1024]

    with SlimTailTileContext(nc) as tc:
        with (
            tc.tile_pool(name="xbp", bufs=1) as xb_pool,
            tc.tile_pool(name="wp", bufs=1) as w_pool,
            tc.tile_pool(name="psum", bufs=8, space="PSUM") as psum_pool,
            tc.tile_pool(name="zout", bufs=4) as z_pool,
            tc.tile_pool(name="warm", bufs=1) as warm_pool,
        ):
            # Only SP / ACT (HWDGE) and gpsimd (SWDGE) can trigger DMAs, and
            # a DGE queue runs its transfers serially, so the W stream (which
            # the PE consumes at ~0.5 us/tile during block 0) is split over
            # two queues in 0.5 MiB pair-DMAs while scalar streams the x
            # row-blocks (needed only every ~4 us).
            wt = [
                w_pool.tile([P, C], bf16, name=f"w{t}", tag=f"w{t}")
                for t in range(T_K)
            ]
            xbt = [
                xb_pool.tile([P, C], bf16, name=f"xb{r}", tag=f"xb{r}")
                for r in range(R_B)
            ]

            def w_dma(eng, t):
                eng.dma_start(wt[t][:], wd[t * P : (t + 1) * P, :])

            def xb_dma(eng, r):
                eng.dma_start(xbt[r][:], xb[r * P : (r + 1) * P, :])

            # Two HWDGE queues (SP, ACT) at ~180 GB/s each, serial per
            # queue; per-tile DMAs so a matmul waits only on the one tile
            # it reads.  W streams on sync at ~1.4 us/tile, which happens
            # to match the PE's early (pstate-ramping) consumption rate of
            # ~1.3 us/tile during block 0; xb streams on scalar and stays
            # well ahead of the one-block-per-4us demand.
            for t in range(T_K):
                w_dma(nc.sync, t)
            for r in range(R_B):
                xb_dma(nc.scalar, r)

            out_eng = [nc.sync, nc.scalar, nc.gpsimd]
            # gpsimd cannot access PSUM; DVE + ACT can (different banks ok)
            evict_eng = [nc.vector.tensor_copy, nc.scalar.copy]
            for r in range(R_B):
                ps = [
                    psum_pool.tile([P, N], mybir.dt.float32, name=f"ps{r}_{h}", tag="ps")
                    for h in range(2)
                ]
                z = z_pool.tile([P, C], bf16, name=f"z{r}", tag="z")
                last = r == R_B - 1
                if not last:
                    # h-inner: consecutive matmuls alternate PSUM banks,
                    # overlapping each matmul's accumulate-drain with the
                    # next one's streaming (same-bank back-to-back matmuls
                    # serialize and cost ~10% PE throughput).
                    for t in range(T_K):
                        for h in range(2):
                            nc.tensor.matmul(
                                ps[h][:],
                                xbt[r][:, t * P : (t + 1) * P],
                                wt[t][:, h * N : (h + 1) * N],
                                start=(t == 0),
                                stop=(t == T_K - 1),
                            )
                    for h in range(2):
                        evict_eng[h](z[:, h * N : (h + 1) * N], ps[h][:])
                    for rep in range(L):
                        out_eng[(r * L + rep) % 3].dma_start(
                            out_rep[r * P : (r + 1) * P, rep, :], z[:]
                        )
                else:
                    # final block h-outer: half 0 evicts and its output
                    # DMAs fly while the PE runs half 1 -- shortens the
                    # kernel tail; keep the tail off the slow gpsimd SWDGE.
                    for h in range(2):
                        for t in range(T_K):
                            nc.tensor.matmul(
                                ps[h][:],
                                xbt[r][:, t * P : (t + 1) * P],
                                wt[t][:, h * N : (h + 1) * N],
                                start=(t == 0),
                                stop=(t == T_K - 1),
                            )
                        evict_eng[h](z[:, h * N : (h + 1) * N], ps[h][:])
                        for rep in range(L):
                            [nc.sync, nc.scalar, nc.scalar if h else nc.sync][rep].dma_start(
                                out_rep[r * P : (r + 1) * P, rep, h * N : (h + 1) * N],
                                z[:, h * N : (h + 1) * N],
                            )

    _split_multiwaits(nc)
    return nc


_NC_CACHE: dict = {}


def _get_nc() -> bass.Bass:
    if "nc" not in _NC_CACHE:
        _NC_CACHE["nc"] = _build()
    return _NC_CACHE["nc"]


def kernel(x_q, x_kv, Wq, Wk, Wv, Wproj):
    B, Tkv, C_ = x_kv.shape
    assert (B, Tkv, C_) == (4, 2048, C)

    # Fold the two projections: z = x @ Wv.T @ Wproj.T = x @ WfT
    WfT = (Wv.astype(np.float64).T @ Wproj.astype(np.float64).T).astype(
        ml_dtypes.bfloat16
    )

    x_flat = x_kv.reshape(B * Tkv, C).astype(ml_dtypes.bfloat16)
    in_maps = []
    for c in range(N_CORES):
        shard = x_flat[c * ROWS_PER_CORE : (c + 1) * ROWS_PER_CORE]
        # [r, row, t, kp] -> [r, kp, t, row]: per row-block, k on partitions
        xb = np.ascontiguousarray(
            shard.reshape(R_B, P, T_K, P).transpose(0, 3, 2, 1)
        ).reshape(ROWS_PER_CORE, C)
        in_maps.append({"xb": xb, "wd": WfT})

    nc = _get_nc()
    res = run_bass_kernel_spmd(nc, in_maps, core_ids=list(range(N_CORES)))

    Tq = L * Tkv
    out_flat = np.concatenate(
        [res.results[c]["out"].astype(np.float32) for c in range(N_CORES)], axis=0
    )  # [B*Tq, C]
    return out_flat.reshape(B, Tq, C)


# revision 13
# speedup vs baseline: 1.2277x; 1.0562x over previous
"""Trainium2 Bass kernel for nn_CrossAttention_28183575396415.

The reference block-mask gives every query exactly one key (kv = q_idx // 3),
so the softmax weight is identically 1 and the q/k projections, RMSNorm and
RoPE are dead code.  The module reduces to

    out[b, t] = x_kv[b, t // 3] @ Wv.T @ Wproj.T
              = x_kv[b, t // 3] @ WfT          with WfT = Wv.T @ Wproj.T

Strategy (8 NeuronCores, SPMD):
  - Host folds the two projection matrices into WfT (float64 accumulate,
    stored bf16) and row-shards the 4*2048 = 8192 kv rows 8 ways (1024
    rows/core).  All device IO is bf16: 4 MiB in + 6 MiB out per core
    (vs 20 MiB for fp32), which moves the kernel from DMA-bound to the
    PE roofline.  rel_l2 error stays ~4e-3, far inside the 2e-2 gate.
  - Device pipeline: the 1024 shard rows are processed as 8 row-blocks of
    128.  Per block: 8 accumulating matmuls per PSUM column-half
    (lhsT = x.T k-tile, stationary; rhs = WfT k-tile, moving), eviction
    PSUM->SBUF with fp32->bf16 downcast, and 3 output DMAs (the t//3
    replication).  Dependencies are block-local, so input DMA, PE, evict
    and output DMA all stream concurrently -- no global barrier.
  - Host unshard = concatenate the 8 bf16 [3072, 1024] slices, upcast to
    fp32.
"""

import json

import numpy as np
import ml_dtypes

import concourse.bass as bass
import concourse.mybir as mybir
from concourse.tile import TileContext
from concourse.vector_clock import ScopedClock
from concourse.bass_utils import run_bass_kernel_spmd

P = 128          # partitions
C = 1024         # model dim
T_K = C // P     # contraction k-tiles
R_B = 8          # row blocks per core (1024 rows / 128)
N = 512          # matmul free dim (one PSUM bank of fp32)
L = 3            # replication factor (Tq // Tkv)
ROWS_PER_CORE = 1024
N_CORES = 8


class SlimTailTileContext(TileContext):
    """Tile's kernel tail is drain -> barrier -> ~280 serialized per-semaphore
    clear instructions -> barrier (~8 us measured).  The clears only matter if
    the loaded NEFF executes more than once; every kernel() call here builds a
    fresh jit executable (fresh NEFF load, semaphores re-initialized), so skip
    them and the second barrier.  The drain still waits for every DMA queue,
    so outputs are complete before the program ends."""

    def _drain_and_barrier(self, tick_clock, wait_clock):
        # The SP drain (with its hoisted wait chain) already gates on every
        # engine's clock and every DMA queue, so outputs are complete when SP
        # retires; with no sem-clears to order, the closing all-engine
        # barrier adds nothing but latency.
        drain_inst = self.nc.sync.drain()
        wait_clock.add_sem_waits(
            drain_inst.ins, ScopedClock({None: tick_clock.global_clock})
        )
        popped = self.nc._tile_sem_poison_stack.pop()
        assert popped is self._sem_poison

def _split_multiwaits(nc: bass.Bass) -> None:
    """This container's walrus allows only ONE sync-wait on several
    instruction formats (Drain/CTRL, Matmult's LDWEIGHTS half, ...).  Tile
    can emit more.  Post-pass the serialized BIR: for any instruction with
    >1 on_wait, hoist all but the last wait onto single-wait EventSemaphore
    carriers inserted immediately before it on the same engine (waits then
    execute in queue order — semantics unchanged).  The patched JSON is
    pinned on the instance so every downstream serialization sees it."""
    raw = bass.Bass.to_json_bytes(nc)
    j = json.loads(raw)
    n_hoisted = 0
    for f in j["functions"]:
        for bb in f["blocks"]:
            new_insts = []
            for ins in bb["instructions"]:
                si = ins.get("sync_info")
                waits = si.get("on_wait", []) if si else []
                if len(waits) > 1:
                    for i, w in enumerate(waits[:-1]):
                        carrier = {
                            "engine": ins["engine"],
                            "ins": [],
                            "outs": [],
                            "name": f"{ins['name']}_hw{i}",
                            "opcode": "EventSemaphore",
                            "sync_info": {"on_update": [], "on_wait": [w]},
                        }
                        if "debug" in ins:
                            carrier["debug"] = ins["debug"]
                        new_insts.append(carrier)
                        n_hoisted += 1
                    si["on_wait"] = waits[-1:]
                new_insts.append(ins)
            bb["instructions"] = new_insts
    patched = json.dumps(j).encode()
    nc.to_json_bytes = lambda: patched


def _build() -> bass.Bass:
    nc = bass.Bass("TRN2")
    bf16 = mybir.dt.bfloat16

    # xb[r*128 + kp, t*128 + row] = x_shard[r*128 + row, t*128 + kp]
    # i.e. per row-block r, the 8 stationary k-tiles side by side.
    xb = nc.dram_tensor("xb", [ROWS_PER_CORE, C], bf16, kind="ExternalInput")
    # wd = WfT  [k, c]
    wd = nc.dram_tensor("wd", [C, C], bf16, kind="ExternalInput")
    out = nc.dram_tensor(
        "out", [L * ROWS_PER_CORE, C], bf16, kind="ExternalOutput"
    )
    # out row (3*g + rep) <- z row g
    out_rep = out.rearrange("(g r) c -> g r c", r=L)  # [1024, 3, 1024]

    with SlimTailTileContext(nc) as tc:
        with (
            tc.tile_pool(name="xbp", bufs=1) as xb_pool,
            tc.tile_pool(name="wp", bufs=1) as w_pool,
            tc.tile_pool(name="psum", bufs=8, space="PSUM") as psum_pool,
            tc.tile_pool(name="zout", bufs=4) as z_pool,
            tc.tile_pool(name="warm", bufs=1) as warm_pool,
        ):
            # Only SP / ACT (HWDGE) and gpsimd (SWDGE) can trigger DMAs, and
            # a DGE queue runs its transfers serially, so the W stream (which
            # the PE consumes at ~0.5 us/tile during block 0) is split over
            # two queues in 0.5 MiB pair-DMAs while scalar streams the x
            # row-blocks (needed only every ~4 us).
            wt = [
                w_pool.tile([P, C], bf16, name=f"w{t}", tag=f"w{t}")
                for t in range(T_K)
            ]
            xbt = [
                xb_pool.tile([P, C], bf16, name=f"xb{r}", tag=f"xb{r}")
                for r in range(R_B)
            ]

            def w_dma(eng, t):
                eng.dma_start(wt[t][:], wd[t * P : (t + 1) * P, :])

            def xb_dma(eng, r):
                eng.dma_start(xbt[r][:], xb[r * P : (r + 1) * P, :])

            # Two HWDGE queues (SP, ACT) at ~180 GB/s each, serial per
            # queue; per-tile DMAs so a matmul waits only on the one tile
            # it reads.  The PE at steady state eats a W tile every
            # ~0.9 us, so W must ride BOTH queues (even t on sync, odd t
            # on scalar behind xb0).  xb1 takes the slow-but-idle gpsimd
            # SWDGE (needed only at ~block-1 time); later xb tiles trail
            # the W stream on both queues, each landing a block early.
            xb_dma(nc.scalar, 0)
            xb_dma(nc.gpsimd, 1)
            for t in range(0, T_K, 2):
                w_dma(nc.sync, t)
            for t in range(1, T_K, 2):
                w_dma(nc.scalar, t)
            for i, r in enumerate(range(2, R_B)):
                xb_dma([nc.sync, nc.scalar][i % 2], r)

            out_eng = [nc.sync, nc.scalar, nc.gpsimd]
            # gpsimd cannot access PSUM; DVE + ACT can (different banks ok)
            evict_eng = [nc.vector.tensor_copy, nc.scalar.copy]
            for r in range(R_B):
                ps = [
                    psum_pool.tile([P, N], mybir.dt.float32, name=f"ps{r}_{h}", tag="ps")
                    for h in range(2)
                ]
                z = z_pool.tile([P, C], bf16, name=f"z{r}", tag="z")
                last = r == R_B - 1
                if not last:
                    # h-inner: consecutive matmuls alternate PSUM banks,
                    # overlapping each matmul's accumulate-drain with the
                    # next one's streaming (same-bank back-to-back matmuls
                    # serialize and cost ~10% PE throughput).
                    for t in range(T_K):
                        for h in range(2):
                            nc.tensor.matmul(
                                ps[h][:],
                                xbt[r][:, t * P : (t + 1) * P],
                                wt[t][:, h * N : (h + 1) * N],
                                start=(t == 0),
                                stop=(t == T_K - 1),
                            )
                    for h in range(2):
                        evict_eng[h](z[:, h * N : (h + 1) * N], ps[h][:])
                    for rep in range(L):
                        out_eng[(r * L + rep) % 3].dma_start(
                            out_rep[r * P : (r + 1) * P, rep, :], z[:]
                        )
                else:
                    # final block h-outer: half 0 evicts and its output
                    # DMAs fly while the PE runs half 1 -- shortens the
                    # kernel tail; keep the tail off the slow gpsimd SWDGE.
                    for h in range(2):
                        for t in range(T_K):
                            nc.tensor.matmul(
                                ps[h][:],
                                xbt[r][:, t * P : (t + 1) * P],
                                wt[t][:, h * N : (h + 1) * N],
                                start=(t == 0),
                                stop=(t == T_K - 1),
                            )
                        evict_eng[h](z[:, h * N : (h + 1) * N], ps[h][:])
                        for rep in range(L):
                            [nc.sync, nc.scalar, nc.scalar if h else nc.sync][rep].dma_start(
                                out_rep[r * P : (r + 1) * P, rep, h * N : (h + 1) * N],
                                z[:, h * N : (h + 1) * N],
                            )

    _split_multiwaits(nc)
    return nc


_NC_CACHE: dict = {}


def _get_nc() -> bass.Bass:
    if "nc" not in _NC_CACHE:
        _NC_CACHE["nc"] = _build()
    return _NC_CACHE["nc"]


def kernel(x_q, x_kv, Wq, Wk, Wv, Wproj):
    B, Tkv, C_ = x_kv.shape
    assert (B, Tkv, C_) == (4, 2048, C)

    # Fold the two projections: z = x @ Wv.T @ Wproj.T = x @ WfT
    WfT = (Wv.astype(np.float64).T @ Wproj.astype(np.float64).T).astype(
        ml_dtypes.bfloat16
    )

    x_flat = x_kv.reshape(B * Tkv, C).astype(ml_dtypes.bfloat16)
    in_maps = []
    for c in range(N_CORES):
        shard = x_flat[c * ROWS_PER_CORE : (c + 1) * ROWS_PER_CORE]
        # [r, row, t, kp] -> [r, kp, t, row]: per row-block, k on partitions
        xb = np.ascontiguousarray(
            shard.reshape(R_B, P, T_K, P).transpose(0, 3, 2, 1)
        ).reshape(ROWS_PER_CORE, C)
        in_maps.append({"xb": xb, "wd": WfT})

    nc = _get_nc()
    res = run_bass_kernel_spmd(nc, in_maps, core_ids=list(range(N_CORES)))

    Tq = L * Tkv
    out_flat = np.concatenate(
        [res.results[c]["out"].astype(np.float32) for c in range(N_CORES)], axis=0
    )  # [B*Tq, C]
    return out_flat.reshape(B, Tq, C)


# revision 15
# speedup vs baseline: 1.2340x; 1.0051x over previous
"""Trainium2 Bass kernel for nn_CrossAttention_28183575396415.

The reference block-mask gives every query exactly one key (kv = q_idx // 3),
so the softmax weight is identically 1 and the q/k projections, RMSNorm and
RoPE are dead code.  The module reduces to

    out[b, t] = x_kv[b, t // 3] @ Wv.T @ Wproj.T
              = x_kv[b, t // 3] @ WfT          with WfT = Wv.T @ Wproj.T

Strategy (8 NeuronCores, SPMD):
  - Host folds the two projection matrices into WfT (float64 accumulate,
    stored bf16) and row-shards the 4*2048 = 8192 kv rows 8 ways (1024
    rows/core).  All device IO is bf16: 4 MiB in + 6 MiB out per core
    (vs 20 MiB for fp32), which moves the kernel from DMA-bound to the
    PE roofline.  rel_l2 error stays ~4e-3, far inside the 2e-2 gate.
  - Device pipeline: the 1024 shard rows are processed as 8 row-blocks of
    128.  Per block: 8 accumulating matmuls per PSUM column-half
    (lhsT = x.T k-tile, stationary; rhs = WfT k-tile, moving), eviction
    PSUM->SBUF with fp32->bf16 downcast, and 3 output DMAs (the t//3
    replication).  Dependencies are block-local, so input DMA, PE, evict
    and output DMA all stream concurrently -- no global barrier.
  - Host unshard = concatenate the 8 bf16 [3072, 1024] slices, upcast to
    fp32.
"""

import json

import numpy as np
import ml_dtypes

import concourse.bass as bass
import concourse.mybir as mybir
from concourse.tile import TileContext
from concourse.vector_clock import ScopedClock
from concourse.bass_utils import run_bass_kernel_spmd

P = 128          # partitions
C = 1024         # model dim
T_K = C // P     # contraction k-tiles
R_B = 8          # row blocks per core (1024 rows / 128)
N = 512          # matmul free dim (one PSUM bank of fp32)
L = 3            # replication factor (Tq // Tkv)
ROWS_PER_CORE = 1024
N_CORES = 8


class SlimTailTileContext(TileContext):
    """Tile's kernel tail is drain -> barrier -> ~280 serialized per-semaphore
    clear instructions -> barrier (~8 us measured).  The clears only matter if
    the loaded NEFF executes more than once; every kernel() call here builds a
    fresh jit executable (fresh NEFF load, semaphores re-initialized), so skip
    them and the second barrier.  The drain still waits for every DMA queue,
    so outputs are complete before the program ends."""

    def _drain_and_barrier(self, tick_clock, wait_clock):
        # The SP drain (with its hoisted wait chain) already gates on every
        # engine's clock and every DMA queue, so outputs are complete when SP
        # retires; with no sem-clears to order, the closing all-engine
        # barrier adds nothing but latency.
        drain_inst = self.nc.sync.drain()
        wait_clock.add_sem_waits(
            drain_inst.ins, ScopedClock({None: tick_clock.global_clock})
        )
        popped = self.nc._tile_sem_poison_stack.pop()
        assert popped is self._sem_poison

def _split_multiwaits(nc: bass.Bass) -> None:
    """This container's walrus allows only ONE sync-wait on several
    instruction formats (Drain/CTRL, Matmult's LDWEIGHTS half, ...).  Tile
    can emit more.  Post-pass the serialized BIR: for any instruction with
    >1 on_wait, hoist all but the last wait onto single-wait EventSemaphore
    carriers inserted immediately before it on the same engine (waits then
    execute in queue order — semantics unchanged).  The patched JSON is
    pinned on the instance so every downstream serialization sees it."""
    raw = bass.Bass.to_json_bytes(nc)
    j = json.loads(raw)
    n_hoisted = 0
    for f in j["functions"]:
        for bb in f["blocks"]:
            new_insts = []
            for ins in bb["instructions"]:
                si = ins.get("sync_info")
                waits = si.get("on_wait", []) if si else []
                if len(waits) > 1:
                    for i, w in enumerate(waits[:-1]):
                        carrier = {
                            "engine": ins["engine"],
                            "ins": [],
                            "outs": [],
                            "name": f"{ins['name']}_hw{i}",
                            "opcode": "EventSemaphore",
                            "sync_info": {"on_update": [], "on_wait": [w]},
                        }
                        if "debug" in ins:
                            carrier["debug"] = ins["debug"]
                        new_insts.append(carrier)
                        n_hoisted += 1
                    si["on_wait"] = waits[-1:]
                new_insts.append(ins)
            bb["instructions"] = new_insts
    patched = json.dumps(j).encode()
    nc.to_json_bytes = lambda: patched


def _build() -> bass.Bass:
    nc = bass.Bass("TRN2")
    bf16 = mybir.dt.bfloat16

    # xb[r*128 + kp, t*128 + row] = x_shard[r*128 + row, t*128 + kp]
    # i.e. per row-block r, the 8 stationary k-tiles side by side.
    xb = nc.dram_tensor("xb", [ROWS_PER_CORE, C], bf16, kind="ExternalInput")
    # wd = WfT  [k, c]
    wd = nc.dram_tensor("wd", [C, C], bf16, kind="ExternalInput")
    out = nc.dram_tensor(
        "out", [L * ROWS_PER_CORE, C], bf16, kind="ExternalOutput"
    )
    # out row (3*g + rep) <- z row g
    out_rep = out.rearrange("(g r) c -> g r c", r=L)  # [1024, 3, 1024]

    with SlimTailTileContext(nc) as tc:
        with (
            tc.tile_pool(name="xbp", bufs=1) as xb_pool,
            tc.tile_pool(name="wp", bufs=1) as w_pool,
            tc.tile_pool(name="psum", bufs=8, space="PSUM") as psum_pool,
            tc.tile_pool(name="zout", bufs=4) as z_pool,
            tc.tile_pool(name="warm", bufs=1) as warm_pool,
        ):
            # Only SP / ACT (HWDGE) and gpsimd (SWDGE) can trigger DMAs, and
            # a DGE queue runs its transfers serially, so the W stream (which
            # the PE consumes at ~0.5 us/tile during block 0) is split over
            # two queues in 0.5 MiB pair-DMAs while scalar streams the x
            # row-blocks (needed only every ~4 us).
            wt = [
                w_pool.tile([P, C], bf16, name=f"w{t}", tag=f"w{t}")
                for t in range(T_K)
            ]
            xbt = [
                xb_pool.tile([P, C], bf16, name=f"xb{r}", tag=f"xb{r}")
                for r in range(R_B)
            ]

            def w_dma(eng, t):
                eng.dma_start(wt[t][:], wd[t * P : (t + 1) * P, :])

            def xb_dma(eng, r):
                eng.dma_start(xbt[r][:], xb[r * P : (r + 1) * P, :])

            # Two HWDGE queues (SP, ACT) at ~180 GB/s each, serial per
            # queue; per-tile DMAs so a matmul waits only on the one tile
            # it reads.  W rides BOTH queues (even t on sync, odd t on
            # scalar); xb0/xb1/xb2 lead their queues so the wide opening
            # phase (blocks 0-2 interleaved over t) can absorb W-arrival
            # jitter; later xb tiles trail, each landing a block early.
            w_dma(nc.sync, 0)
            xb_dma(nc.scalar, 0)
            xb_dma(nc.sync, 1)
            xb_dma(nc.scalar, 2)
            for t in range(2, T_K, 2):
                w_dma(nc.sync, t)
            for t in range(1, T_K, 2):
                w_dma(nc.scalar, t)
            for i, r in enumerate(range(3, R_B)):
                xb_dma([nc.sync, nc.scalar][i % 2], r)

            out_eng = [nc.sync, nc.scalar, nc.gpsimd]
            # gpsimd cannot access PSUM; DVE + ACT can (different banks ok)
            evict_eng = [nc.vector.tensor_copy, nc.scalar.copy]

            # Wide opening: blocks 0-2 interleaved over t across 6 PSUM
            # banks.  The t order tracks DMA arrival (evens on sync, odds
            # on scalar), and any landed W tile feeds 6 matmuls, so the PE
            # stays busy through the W stream instead of stalling on the
            # strict t-order of a single block.  PSUM accumulation order
            # is free; start/stop flag the first/last write per bank.
            WIDE = 3
            tt_order = [0, 2, 1, 4, 3, 6, 5, 7]
            wide_ps = {
                (b, h): psum_pool.tile(
                    [P, N], mybir.dt.float32, name=f"ps{b}_{h}", tag="ps"
                )
                for b in range(WIDE)
                for h in range(2)
            }
            for pos, t in enumerate(tt_order):
                for b in range(WIDE):
                    for h in range(2):
                        nc.tensor.matmul(
                            wide_ps[(b, h)][:],
                            xbt[b][:, t * P : (t + 1) * P],
                            wt[t][:, h * N : (h + 1) * N],
                            start=(pos == 0),
                            stop=(pos == T_K - 1),
                        )
            for b in range(WIDE):
                z = z_pool.tile([P, C], bf16, name=f"z{b}", tag="z")
                for h in range(2):
                    evict_eng[h](z[:, h * N : (h + 1) * N], wide_ps[(b, h)][:])
                for rep in range(L):
                    out_eng[(b * L + rep) % 3].dma_start(
                        out_rep[b * P : (b + 1) * P, rep, :], z[:]
                    )

            for r in range(WIDE, R_B):
                ps = [
                    psum_pool.tile([P, N], mybir.dt.float32, name=f"ps{r}_{h}", tag="ps")
                    for h in range(2)
                ]
                z = z_pool.tile([P, C], bf16, name=f"z{r}", tag="z")
                last = r == R_B - 1
                if not last:
                    # h-inner: consecutive matmuls alternate PSUM banks,
                    # overlapping each matmul's accumulate-drain with the
                    # next one's streaming (same-bank back-to-back matmuls
                    # serialize and cost ~10% PE throughput).
                    for t in range(T_K):
                        for h in range(2):
                            nc.tensor.matmul(
                                ps[h][:],
                                xbt[r][:, t * P : (t + 1) * P],
                                wt[t][:, h * N : (h + 1) * N],
                                start=(t == 0),
                                stop=(t == T_K - 1),
                            )
                    for h in range(2):
                        evict_eng[h](z[:, h * N : (h + 1) * N], ps[h][:])
                    for rep in range(L):
                        out_eng[(r * L + rep) % 3].dma_start(
                            out_rep[r * P : (r + 1) * P, rep, :], z[:]
                        )
                else:
                    # final block h-outer: half 0 evicts and its output
                    # DMAs fly while the PE runs half 1 -- shortens the
                    # kernel tail; keep the tail off the slow gpsimd SWDGE.
                    for h in range(2):
                        for t in range(T_K):
                            nc.tensor.matmul(
                                ps[h][:],
                                xbt[r][:, t * P : (t + 1) * P],
                                wt[t][:, h * N : (h + 1) * N],
                                start=(t == 0),
                                stop=(t == T_K - 1),
                            )
                        evict_eng[h](z[:, h * N : (h + 1) * N], ps[h][:])
                        for rep in range(L):
                            [nc.sync, nc.scalar, nc.scalar if h else nc.sync][rep].dma_start(
                                out_rep[r * P : (r + 1) * P, rep, h * N : (h + 1) * N],
                                z[:, h * N : (h + 1) * N],
                            )

    _split_multiwaits(nc)
    return nc


_NC_CACHE: dict = {}


def _get_nc() -> bass.Bass:
    if "nc" not in _NC_CACHE:
        _NC_CACHE["nc"] = _build()
    return _NC_CACHE["nc"]


def kernel(x_q, x_kv, Wq, Wk, Wv, Wproj):
    B, Tkv, C_ = x_kv.shape
    assert (B, Tkv, C_) == (4, 2048, C)

    # Fold the two projections: z = x @ Wv.T @ Wproj.T = x @ WfT
    WfT = (Wv.astype(np.float64).T @ Wproj.astype(np.float64).T).astype(
        ml_dtypes.bfloat16
    )

    x_flat = x_kv.reshape(B * Tkv, C).astype(ml_dtypes.bfloat16)
    in_maps = []
    for c in range(N_CORES):
        shard = x_flat[c * ROWS_PER_CORE : (c + 1) * ROWS_PER_CORE]
        # [r, row, t, kp] -> [r, kp, t, row]: per row-block, k on partitions
        xb = np.ascontiguousarray(
            shard.reshape(R_B, P, T_K, P).transpose(0, 3, 2, 1)
        ).reshape(ROWS_PER_CORE, C)
        in_maps.append({"xb": xb, "wd": WfT})

    nc = _get_nc()
    res = run_bass_kernel_spmd(nc, in_maps, core_ids=list(range(N_CORES)))

    Tq = L * Tkv
    out_flat = np.concatenate(
        [res.results[c]["out"].astype(np.float32) for c in range(N_CORES)], axis=0
    )  # [B*Tq, C]
    return out_flat.reshape(B, Tq, C)
